# revision 11
# baseline (speedup 1.0000x reference)
"""NT-Xent loss on 8 Trainium2 NeuronCores (Bass/Tile), v7.

Reference computation (B=4096, D=1024, T=0.5):
    x  = concat(z_i, z_j)                      # [8192, 1024] f32
    xn = x / ||x||                             # row-normalize
    sim = xn @ xn.T                            # [8192, 8192]
    logits = sim / T, diag masked to -inf
    loss = -mean(log_softmax(logits)[i, target(i)]), target(i) = i ^ 1

E = exp(sim/T) is symmetric, so only half the matrix need be computed.
Core c owns rows [1024c, 1024(c+1)) and sweeps ~4.25 of the 8 column
blocks:
  - blocks c+1..c+3 (mod 8) in full,
  - its own diagonal block minus the lower-left quadrant (the triangle
    trick: quadrant D10 is recovered from a column-sum of D01
    restricted to the top m-half),
  - half of block c+4, quadrant-split between the two endpoint cores
    (cores 0-3 take the diagonal quadrants of their d=4 block, cores
    4-7 the anti-diagonal)
-- together an exact single cover.  Row sums come from the ACT exp
accumulator (wide phases) or DVE reduces (1-chunk phases); transposed
halves are covered by DVE column-sum accumulators shipped to the host.
The host combines partials, subtracts the diagonal, and takes
mean(log denom - log E_target) in f64 -- an O(N) numpy epilogue.

Normalization happens on the host (O(N*D) staging prep, like the
transpose + fp8 casts); both matmul sides are fp8 at scale 16, sliced
from the same chunk tiles (the d=0 chunks ARE the own rows), so the
device is a pure DoubleRow-fp8 sweep (PE) + one wide exp per PSUM tile
(ACT) + bf16 colsum adds / rowsum reduces / diag+target extraction
(DVE).

Phase structure (chunk-count-increasing so activation width only
grows and each phase's rhs lands during earlier phases):
  A: chunk 0, m 0-3 as one quad     (one [128,2048] exp)
  B: chunk 1, two m-quads
  E: chunk 8/9 by m-half, two quads
  C: chunks 2-3 per m               ([128,1024] exps, ACT accum)
  D1: chunks 4-5 per m
  D2: chunks 6-7 per m
1-chunk phases batch 4 m-tiles into one 4-bank PSUM tile so the ACT
chain (one exp per quad) never gates the PSUM ring (v6 lesson: per-m
512-wide exps + accumulator reads ran neck-and-neck with the 4-matmul
m-groups and hiccuped the PE every other m-tile).  Each colsum
accumulator DMAs out as soon as its last add retires; dummy warm-up
matmuls hold the HAM clock-gate at 2.4 GHz through the DMA prologue.
"""

import numpy as np
import ml_dtypes
from contextlib import ExitStack

import concourse.bass as bass
import concourse.tile as tile
from concourse import bacc, mybir
from concourse.bass_utils import run_bass_kernel_spmd

F32 = mybir.dt.float32
BF16 = mybir.dt.bfloat16
F8 = mybir.dt.float8e4
BF = ml_dtypes.bfloat16
F8NP = ml_dtypes.float8_e4m3
F8SCALE = 16.0
EXPSCALE = 2.0 / (F8SCALE * F8SCALE)   # exp((16 xn_r . 16 xn_c) / (256 T))

B = 4096
D = 1024
N = 2 * B            # 8192 rows total
NCORES = 8
RPC = N // NCORES    # 1024 rows per core
KT = D // 128        # 8 contraction partition-tiles
MT = RPC // 128      # 8 row tiles per core
CHUNK = 512
NREG = 4 * RPC       # regular swept columns (blocks d=0..3)
NG2 = RPC            # staged d=4 columns (chunk 8 for m<4, chunk 9 for m>=4)
NCOL = NREG + NG2    # 5120 staged columns
NCH = NCOL // CHUNK  # 10 column chunks
NCS = 6              # chunks whose colsums ship (d=1..3)
NWARM = 9            # dummy PE warm-up matmuls during the DMA prologue

# phases: (chunk list, m-groups); -1 means chunk 8 or 9 by m-half.
# 1-chunk phases use m-quads (4 m-tiles per PSUM tile); wide phases one
# m per tile.  Phase A covers only the top m-half of chunk 0 (triangle).
PHASES = [
    ([0], [[0, 1, 2, 3]]),
    ([1], [[0, 1, 2, 3], [4, 5, 6, 7]]),
    ([-1], [[0, 1, 2, 3], [4, 5, 6, 7]]),
    ([2, 3], [[m] for m in range(MT)]),
    ([4, 5], [[m] for m in range(MT)]),
    ([6, 7], [[m] for m in range(MT)]),
]
NPH = len(PHASES)

_NC_CACHE = {}
LAST_RESULTS = None  # BassKernelResults of the most recent run (for test.py)


def _build_program():
    nc = bacc.Bacc("TRN2", target_bir_lowering=False, debug=False)

    xall8 = nc.dram_tensor("xall8", [NCH, 128, KT, CHUNK], F8, kind="ExternalInput")
    masks = nc.dram_tensor("masks", [128, 256], BF16, kind="ExternalInput")
    esum_out = nc.dram_tensor("esum", [128, MT, NPH], F32, kind="ExternalOutput")
    ediag_out = nc.dram_tensor("ediag", [128, MT], F32, kind="ExternalOutput")
    etarg_out = nc.dram_tensor("etarg", [128, MT], F32, kind="ExternalOutput")
    # [cs13 d=1..3 | cs4a | cs4b | cs_d01]
    cs_out = nc.dram_tensor("colsums", [128, NCS * CHUNK + NG2 + CHUNK], BF16,
                            kind="ExternalOutput")

    ADD = mybir.AluOpType.add
    EXP = mybir.ActivationFunctionType.Exp

    with tile.TileContext(nc) as tc, ExitStack() as ctx:
        consts = ctx.enter_context(tc.tile_pool(name="consts", bufs=1))
        xin_pool = ctx.enter_context(tc.tile_pool(name="xin", bufs=1))
        exp_pool = ctx.enter_context(tc.tile_pool(name="exp", bufs=3))
        scr_pool = ctx.enter_context(tc.tile_pool(name="scr", bufs=2))
        stat_pool = ctx.enter_context(tc.tile_pool(name="stat", bufs=1))
        ps_pool = ctx.enter_context(tc.tile_pool(name="ps", bufs=2, space="PSUM"))
        # 2 x 4-bank PSUM tiles == all 8 banks

        # PE warm-up operand: memset'd zeros, no DMA dependency
        zbf = consts.tile([128, 640], BF16)
        nc.vector.memset(zbf[:], 0.0)

        mask_sb = consts.tile([128, 256], BF16)
        xc = [xin_pool.tile([128, KT, CHUNK], F8, name=f"xc{j}")
              for j in range(NCH)]

        # DMA issue order by first use: chunk 0 split across the three
        # DMA-capable queues, chunk 1 halved, then chunks 8/9 (phase E
        # is third), then the rest round-robin.
        nc.scalar.dma_start(xc[0][:, 0:3], xall8[0, :, 0:3])
        nc.sync.dma_start(xc[0][:, 3:6], xall8[0, :, 3:6])
        nc.gpsimd.dma_start(xc[0][:, 6:KT], xall8[0, :, 6:KT])
        h = KT // 2
        nc.scalar.dma_start(xc[1][:, 0:h], xall8[1, :, 0:h])
        nc.sync.dma_start(xc[1][:, h:KT], xall8[1, :, h:KT])
        nc.gpsimd.dma_start(mask_sb[:], masks[:])
        nc.gpsimd.dma_start(xc[8][:], xall8[8])
        nc.scalar.dma_start(xc[9][:], xall8[9])
        nc.sync.dma_start(xc[2][:], xall8[2])
        nc.gpsimd.dma_start(xc[3][:], xall8[3])
        nc.scalar.dma_start(xc[4][:], xall8[4])
        nc.sync.dma_start(xc[5][:], xall8[5])
        nc.gpsimd.dma_start(xc[6][:], xall8[6])
        nc.scalar.dma_start(xc[7][:], xall8[7])

        esum = stat_pool.tile([128, MT, NPH], F32)
        ediag = stat_pool.tile([128, MT], F32)
        etarg = stat_pool.tile([128, MT], F32)
        cs13 = stat_pool.tile([128, NCS * CHUNK], BF16)
        cs4a = stat_pool.tile([128, CHUNK], BF16)
        cs4b = stat_pool.tile([128, CHUNK], BF16)
        csd01 = stat_pool.tile([128, CHUNK], BF16)
        nc.vector.memset(esum[:], 0.0)
        nc.vector.memset(cs13[:], 0.0)
        nc.vector.memset(cs4a[:], 0.0)
        nc.vector.memset(cs4b[:], 0.0)
        nc.vector.memset(csd01[:], 0.0)

        # HAM warm-up: dummy bf16 matmuls fill the otherwise-idle PE
        # during the first chunk's DMA so the 2.4 GHz clock-gate opens
        # before the real sweep starts.
        ps_w = ps_pool.tile([128, 2048], F32, name="ps", tag="ps")
        for _ in range(NWARM):
            nc.tensor.matmul(ps_w[:, 0:CHUNK], lhsT=zbf[:, 0:128],
                             rhs=zbf[:, 128:640], start=True, stop=True)

        def extract(esb, m, col):
            """diag + target extraction for m-tile m at esb column col."""
            scr = scr_pool.tile([128, 128], BF16)
            nc.vector.tensor_mul(scr[:], esb[:, col:col + 128],
                                 mask_sb[:, 0:128])
            nc.vector.tensor_reduce(ediag[:, m:m + 1], scr[:],
                                    axis=mybir.AxisListType.X, op=ADD)
            scr2 = scr_pool.tile([128, 128], BF16)
            nc.vector.tensor_mul(scr2[:], esb[:, col:col + 128],
                                 mask_sb[:, 128:256])
            nc.vector.tensor_reduce(etarg[:, m:m + 1], scr2[:],
                                    axis=mybir.AxisListType.X, op=ADD)

        for ph, (chunks, mgroups) in enumerate(PHASES):
            for mg in mgroups:
                quad = len(mg) > 1
                nch = len(chunks)
                w = CHUNK * (len(mg) if quad else nch)
                ps = ps_pool.tile([128, 2048], F32, name="ps", tag="ps")
                for idx, m in enumerate(mg):
                    cj = [(8 if m < 4 else 9) if j < 0 else j for j in chunks]
                    for t in range(KT // 2):
                        for ci, j in enumerate(cj):
                            bank = idx if quad else ci
                            nc.tensor.matmul(
                                ps[:, CHUNK * bank:CHUNK * (bank + 1)],
                                lhsT=xc[m // 4][:, 2 * t:2 * t + 2,
                                                128 * (m % 4):128 * (m % 4) + 128],
                                rhs=xc[j][:, 2 * t:2 * t + 2, :],
                                start=(t == 0), stop=(t == KT // 2 - 1),
                                perf_mode=mybir.MatmulPerfMode.DoubleRow,
                            )
                esb = exp_pool.tile([128, w], BF16)
                if quad:
                    nc.scalar.activation(esb[:], ps[:, 0:w], EXP,
                                         scale=EXPSCALE)
                    for idx, m in enumerate(mg):
                        nc.vector.tensor_reduce(
                            esum[:, m, ph:ph + 1],
                            esb[:, CHUNK * idx:CHUNK * (idx + 1)],
                            axis=mybir.AxisListType.X, op=ADD)
                else:
                    m = mg[0]
                    nc.scalar.activation(esb[:], ps[:, 0:w], EXP,
                                         scale=EXPSCALE,
                                         accum_out=esum[:, m, ph:ph + 1])
                for idx, m in enumerate(mg):
                    base = CHUNK * idx if quad else 0
                    if ph == 0:
                        extract(esb, m, base + 128 * m)
                    elif ph == 1:
                        if m >= 4:
                            extract(esb, m, base + 128 * m - CHUNK)
                        else:
                            nc.vector.tensor_add(
                                csd01[:], csd01[:],
                                esb[:, base:base + CHUNK])
                    elif ph == 2:
                        tgt = cs4a if m < 4 else cs4b
                        nc.vector.tensor_add(tgt[:], tgt[:],
                                             esb[:, base:base + CHUNK])
                    elif ph == 3:
                        nc.vector.tensor_add(cs13[:, 0:1024],
                                             cs13[:, 0:1024], esb[:])
                    elif ph == 4:
                        nc.vector.tensor_add(cs13[:, 1024:2048],
                                             cs13[:, 1024:2048], esb[:])
                    elif ph == 5:
                        nc.vector.tensor_add(cs13[:, 2048:3072],
                                             cs13[:, 2048:3072], esb[:])
            # ship each accumulator as soon as its last add retires so
            # only the final phase's piece remains for the tail
            s = NCS * CHUNK
            if ph == 1:
                nc.gpsimd.dma_start(cs_out[:, s + 2 * CHUNK:], csd01[:])
                nc.gpsimd.dma_start(ediag_out[:], ediag[:])
                nc.gpsimd.dma_start(etarg_out[:], etarg[:])
            elif ph == 2:
                nc.gpsimd.dma_start(cs_out[:, s:s + CHUNK], cs4a[:])
                nc.gpsimd.dma_start(cs_out[:, s + CHUNK:s + 2 * CHUNK],
                                    cs4b[:])
            elif ph == 3:
                nc.gpsimd.dma_start(cs_out[:, 0:1024], cs13[:, 0:1024])
            elif ph == 4:
                nc.gpsimd.dma_start(cs_out[:, 1024:2048], cs13[:, 1024:2048])

        nc.sync.dma_start(cs_out[:, 2048:2560], cs13[:, 2048:2560])
        nc.gpsimd.dma_start(cs_out[:, 2560:3072], cs13[:, 2560:3072])
        nc.sync.dma_start(esum_out[:], esum[:])

    nc.finalize()
    return nc


def _get_program():
    if "nc" not in _NC_CACHE:
        _NC_CACHE["nc"] = _build_program()
    return _NC_CACHE["nc"]


def _make_masks():
    m = np.zeros((128, 256), dtype=np.float32)
    p = np.arange(128)
    m[p, p] = 1.0              # identity: diagonal extraction
    m[p, 128 + (p ^ 1)] = 1.0  # pair-swap: target extraction
    return m.astype(BF)


def kernel(z_i: np.ndarray, z_j: np.ndarray, _trace: bool = False) -> np.ndarray:
    global LAST_RESULTS
    nc = _get_program()

    x = np.concatenate([np.asarray(z_i), np.asarray(z_j)], axis=0)
    assert x.shape == (N, D) and x.dtype == np.float32
    xn = x / np.maximum(np.sqrt((x.astype(np.float64) ** 2).sum(axis=1,
                        keepdims=True)), 1e-8)
    x8 = (xn * F8SCALE).astype(F8NP)             # [8192, 1024] fp8
    x8t = np.ascontiguousarray(x8.T)             # [1024, 8192] fp8
    masks = _make_masks()

    in_maps = []
    for c in range(NCORES):
        b = ((c + 4) % NCORES) * RPC
        if c < 4:
            g2cols = b + np.arange(NG2)
        else:
            g2cols = b + (np.arange(NG2) + 512) % NG2
        cols = np.concatenate([(c * RPC + np.arange(NREG)) % N, g2cols])
        # chunk-major staging: [NCH, 128, KT, CHUNK], contiguous per
        # (chunk, partition) so each chunk DMA is one 4KB run/partition
        xt_c = x8t[:, cols].reshape(KT, 128, NCH, CHUNK)
        xall_c = np.ascontiguousarray(xt_c.transpose(2, 1, 0, 3))
        in_maps.append({"xall8": xall_c, "masks": masks})

    res = run_bass_kernel_spmd(
        nc, in_maps, core_ids=list(range(NCORES)), trace=_trace,
    )
    LAST_RESULTS = res

    # Host epilogue (O(N) numpy, f64): combine row partials with the
    # symmetric colsum partials, then mean(log denom - log E_target).
    denom = np.zeros(N, dtype=np.float64)
    ediag = np.zeros(N, dtype=np.float64)
    etarg = np.zeros(N, dtype=np.float64)
    pm = (128 * np.arange(MT)[None, :] + np.arange(128)[:, None]).ravel()
    for c in range(NCORES):
        r = res.results[c]
        rows = RPC * c + pm
        denom[rows] += r["esum"].astype(np.float64).sum(axis=2).ravel()
        ediag[rows] = r["ediag"].astype(np.float64).ravel()
        etarg[rows] = r["etarg"].astype(np.float64).ravel()
        cs = r["colsums"].astype(np.float64).sum(axis=0)
        s = NCS * CHUNK
        gcols = (RPC * c + RPC + np.arange(NCS * CHUNK)) % N
        denom[gcols] += cs[0:s]
        b = ((c + 4) % NCORES) * RPC
        if c < 4:
            g2cols = b + np.arange(NG2)
        else:
            g2cols = b + (np.arange(NG2) + 512) % NG2
        denom[g2cols[0:CHUNK]] += cs[s:s + CHUNK]
        denom[g2cols[CHUNK:]] += cs[s + CHUNK:s + NG2]
        denom[c * RPC + CHUNK + np.arange(CHUNK)] += cs[s + NG2:]
    loss = np.mean(np.log(denom - ediag) - np.log(etarg))
    return np.float32(loss)


# revision 14
# speedup vs baseline: 1.0520x; 1.0520x over previous
"""NT-Xent loss on 8 Trainium2 NeuronCores (Bass/Tile), v8.

Reference computation (B=4096, D=1024, T=0.5):
    x  = concat(z_i, z_j)                      # [8192, 1024] f32
    xn = x / ||x||                             # row-normalize
    sim = xn @ xn.T                            # [8192, 8192]
    logits = sim / T, diag masked to -inf
    loss = -mean(log_softmax(logits)[i, target(i)]), target(i) = i ^ 1

The O(N^2 D) work is the softmax DENOMINATOR (row sums of
E = exp(sim/T)); that is what the device computes.  The numerator
E[i, i^1] and the diagonal correction E[i, i] are 8192 dot products
each -- O(N D) -- computed on the host from the very same fp8 operands
the device multiplies, so they agree with the device's matrix entries
to float32 rounding.

E is symmetric, so only half the matrix need be computed.  Core c owns
rows [1024c, 1024(c+1)) and sweeps ~4.25 of the 8 column blocks:
  - blocks c+1..c+3 (mod 8) in full,
  - its own diagonal block minus the lower-left quadrant (the triangle
    trick: quadrant D10 is recovered from a column-sum of D01
    restricted to the top m-half),
  - half of block c+4, quadrant-split between the two endpoint cores
    (cores 0-3 take the diagonal quadrants of their d=4 block, cores
    4-7 the anti-diagonal)
-- together an exact single cover.  Row sums come from the ACT exp
accumulator (wide phases) or GpSimd reduces (1-chunk phases);
transposed halves are covered by DVE column-sum accumulators shipped
to the host.  The host combines partials in f64: an O(N) epilogue.

Normalization happens on the host (O(N*D) staging prep, like the
transpose + fp8 casts); both matmul sides are fp8 at scale 16, sliced
from the same chunk tiles (the d=0 chunks ARE the own rows), so the
device is a pure DoubleRow-fp8 sweep (PE) + one wide exp per PSUM tile
(ACT) + bf16 colsum adds (DVE) + rowsum reduces (GpSimd).

Phase structure (chunk-count-increasing so each phase's rhs lands
during earlier phases):
  A: chunk 0, m 0-3 as one quad     (one [128,2048] exp)
  B: chunk 1, two m-quads
  E: chunk 8/9 by m-half, two quads
  C: chunks 2-3 per m               ([128,1024] exps, ACT accum)
  D1: chunks 4-5 per m
  D2: chunks 6-7 per m
1-chunk phases batch 4 m-tiles into one 4-bank PSUM tile so the ACT
chain never gates the PSUM ring; their per-m rowsums cannot use the
ACT accumulator (it would mix the quad's rows), so GpSimd -- otherwise
idle -- reduces each esb quarter (v7 lesson: those reduces plus the
diag/target extraction overloaded the DVE, backed up the 3-deep esb
ring, and stalled ACT -> PSUM -> PE for 3.6 us; v8 drops extraction
entirely, moves reduces to GpSimd, and deepens the esb ring).  Each
colsum accumulator DMAs out as soon as its last add retires; dummy
warm-up matmuls hold the HAM clock-gate at 2.4 GHz through the DMA
prologue.
"""

import numpy as np
import ml_dtypes
from contextlib import ExitStack

import concourse.bass as bass
import concourse.tile as tile
from concourse import bacc, mybir
from concourse.bass_utils import run_bass_kernel_spmd

F32 = mybir.dt.float32
BF16 = mybir.dt.bfloat16
F8 = mybir.dt.float8e4
BF = ml_dtypes.bfloat16
F8NP = ml_dtypes.float8_e4m3
F8SCALE = 16.0
EXPSCALE = 2.0 / (F8SCALE * F8SCALE)   # exp((16 xn_r . 16 xn_c) / (256 T))

B = 4096
D = 1024
N = 2 * B            # 8192 rows total
NCORES = 8
RPC = N // NCORES    # 1024 rows per core
KT = D // 128        # 8 contraction partition-tiles
MT = RPC // 128      # 8 row tiles per core
CHUNK = 512
NREG = 4 * RPC       # regular swept columns (blocks d=0..3)
NG2 = RPC            # staged d=4 columns (chunk 8 for m<4, chunk 9 for m>=4)
NCOL = NREG + NG2    # 5120 staged columns
NCH = NCOL // CHUNK  # 10 column chunks
NCS = 6              # chunks whose colsums ship (d=1..3)
NWARM = 9            # dummy PE warm-up matmuls during the DMA prologue

# phases: (chunk list, m-groups); -1 means chunk 8 or 9 by m-half.
# 1-chunk phases use m-quads (4 m-tiles per PSUM tile); wide phases one
# m per tile.  Phase A covers only the top m-half of chunk 0 (triangle).
PHASES = [
    ([0], [[0, 1, 2, 3]]),
    ([1], [[0, 1, 2, 3], [4, 5, 6, 7]]),
    ([-1], [[0, 1, 2, 3], [4, 5, 6, 7]]),
    ([2, 3], [[m] for m in range(MT)]),
    ([4, 5], [[m] for m in range(MT)]),
    ([6, 7], [[m] for m in range(MT)]),
]
NPH = len(PHASES)

_NC_CACHE = {}
LAST_RESULTS = None  # BassKernelResults of the most recent run (for test.py)


def _build_program():
    nc = bacc.Bacc("TRN2", target_bir_lowering=False, debug=False)

    xall8 = nc.dram_tensor("xall8", [NCH, 128, KT, CHUNK], F8, kind="ExternalInput")
    esum_out = nc.dram_tensor("esum", [128, MT, NPH], F32, kind="ExternalOutput")
    # [cs13 d=1..3 | cs4a | cs4b | cs_d01]
    cs_out = nc.dram_tensor("colsums", [128, NCS * CHUNK + NG2 + CHUNK], BF16,
                            kind="ExternalOutput")

    ADD = mybir.AluOpType.add
    EXP = mybir.ActivationFunctionType.Exp

    with tile.TileContext(nc) as tc, ExitStack() as ctx:
        consts = ctx.enter_context(tc.tile_pool(name="consts", bufs=1))
        xin_pool = ctx.enter_context(tc.tile_pool(name="xin", bufs=1))
        exp_pool = ctx.enter_context(tc.tile_pool(name="exp", bufs=8))
        stat_pool = ctx.enter_context(tc.tile_pool(name="stat", bufs=1))
        ps_pool = ctx.enter_context(tc.tile_pool(name="ps", bufs=2, space="PSUM"))
        # 2 x 4-bank PSUM tiles == all 8 banks

        # PE warm-up operand: memset'd zeros, no DMA dependency
        zbf = consts.tile([128, 640], BF16)
        nc.vector.memset(zbf[:], 0.0)

        xc = [xin_pool.tile([128, KT, CHUNK], F8, name=f"xc{j}")
              for j in range(NCH)]

        # DMA issue order by first use: chunk 0 split across the three
        # DMA-capable queues, chunk 1 halved, then chunks 8/9 (phase E
        # is third), then the rest round-robin.
        nc.scalar.dma_start(xc[0][:, 0:3], xall8[0, :, 0:3])
        nc.sync.dma_start(xc[0][:, 3:6], xall8[0, :, 3:6])
        nc.gpsimd.dma_start(xc[0][:, 6:KT], xall8[0, :, 6:KT])
        h = KT // 2
        nc.scalar.dma_start(xc[1][:, 0:h], xall8[1, :, 0:h])
        nc.sync.dma_start(xc[1][:, h:KT], xall8[1, :, h:KT])
        nc.gpsimd.dma_start(xc[8][:], xall8[8])
        nc.scalar.dma_start(xc[9][:], xall8[9])
        nc.sync.dma_start(xc[2][:], xall8[2])
        nc.gpsimd.dma_start(xc[3][:], xall8[3])
        nc.scalar.dma_start(xc[4][:], xall8[4])
        nc.sync.dma_start(xc[5][:], xall8[5])
        nc.gpsimd.dma_start(xc[6][:], xall8[6])
        nc.scalar.dma_start(xc[7][:], xall8[7])

        esum = stat_pool.tile([128, MT, NPH], F32)
        cs13 = stat_pool.tile([128, NCS * CHUNK], BF16)
        cs4a = stat_pool.tile([128, CHUNK], BF16)
        cs4b = stat_pool.tile([128, CHUNK], BF16)
        csd01 = stat_pool.tile([128, CHUNK], BF16)
        nc.vector.memset(esum[:], 0.0)
        nc.vector.memset(cs13[:], 0.0)
        nc.vector.memset(cs4a[:], 0.0)
        nc.vector.memset(cs4b[:], 0.0)
        nc.vector.memset(csd01[:], 0.0)

        # HAM warm-up: dummy bf16 matmuls fill the otherwise-idle PE
        # during the first chunk's DMA so the 2.4 GHz clock-gate opens
        # before the real sweep starts.
        ps_w = ps_pool.tile([128, 2048], F32, name="ps", tag="ps")
        for _ in range(NWARM):
            nc.tensor.matmul(ps_w[:, 0:CHUNK], lhsT=zbf[:, 0:128],
                             rhs=zbf[:, 128:640], start=True, stop=True)

        for ph, (chunks, mgroups) in enumerate(PHASES):
            for mg in mgroups:
                quad = len(mg) > 1
                nch = len(chunks)
                w = CHUNK * (len(mg) if quad else nch)
                ps = ps_pool.tile([128, 2048], F32, name="ps", tag="ps")
                for idx, m in enumerate(mg):
                    cj = [(8 if m < 4 else 9) if j < 0 else j for j in chunks]
                    for t in range(KT // 2):
                        for ci, j in enumerate(cj):
                            bank = idx if quad else ci
                            nc.tensor.matmul(
                                ps[:, CHUNK * bank:CHUNK * (bank + 1)],
                                lhsT=xc[m // 4][:, 2 * t:2 * t + 2,
                                                128 * (m % 4):128 * (m % 4) + 128],
                                rhs=xc[j][:, 2 * t:2 * t + 2, :],
                                start=(t == 0), stop=(t == KT // 2 - 1),
                                perf_mode=mybir.MatmulPerfMode.DoubleRow,
                            )
                esb = exp_pool.tile([128, w], BF16)
                if quad:
                    nc.scalar.activation(esb[:], ps[:, 0:w], EXP,
                                         scale=EXPSCALE)
                    for idx, m in enumerate(mg):
                        nc.vector.tensor_reduce(
                            esum[:, m, ph:ph + 1],
                            esb[:, CHUNK * idx:CHUNK * (idx + 1)],
                            axis=mybir.AxisListType.X, op=ADD)
                else:
                    m = mg[0]
                    nc.scalar.activation(esb[:], ps[:, 0:w], EXP,
                                         scale=EXPSCALE,
                                         accum_out=esum[:, m, ph:ph + 1])
                for idx, m in enumerate(mg):
                    base = CHUNK * idx if quad else 0
                    if ph == 1 and m < 4:
                        nc.gpsimd.tensor_add(csd01[:], csd01[:],
                                             esb[:, base:base + CHUNK])
                    elif ph == 2:
                        tgt = cs4a if m < 4 else cs4b
                        nc.gpsimd.tensor_add(tgt[:], tgt[:],
                                             esb[:, base:base + CHUNK])
                    elif ph == 3:
                        nc.vector.tensor_add(cs13[:, 0:1024],
                                             cs13[:, 0:1024], esb[:])
                    elif ph == 4:
                        nc.vector.tensor_add(cs13[:, 1024:2048],
                                             cs13[:, 1024:2048], esb[:])
                    elif ph == 5:
                        nc.vector.tensor_add(cs13[:, 2048:3072],
                                             cs13[:, 2048:3072], esb[:])
            # ship each accumulator as soon as its last add retires so
            # only the final phase's piece remains for the tail
            s = NCS * CHUNK
            if ph == 1:
                nc.sync.dma_start(cs_out[:, s + 2 * CHUNK:], csd01[:])
            elif ph == 2:
                nc.sync.dma_start(cs_out[:, s:s + CHUNK], cs4a[:])
                nc.sync.dma_start(cs_out[:, s + CHUNK:s + 2 * CHUNK],
                                  cs4b[:])
            elif ph == 3:
                nc.sync.dma_start(cs_out[:, 0:1024], cs13[:, 0:1024])
            elif ph == 4:
                nc.sync.dma_start(cs_out[:, 1024:2048], cs13[:, 1024:2048])

        nc.sync.dma_start(cs_out[:, 2048:2560], cs13[:, 2048:2560])
        nc.scalar.dma_start(cs_out[:, 2560:3072], cs13[:, 2560:3072])
        nc.sync.dma_start(esum_out[:], esum[:])

    nc.finalize()
    return nc


def _get_program():
    if "nc" not in _NC_CACHE:
        _NC_CACHE["nc"] = _build_program()
    return _NC_CACHE["nc"]


def kernel(z_i: np.ndarray, z_j: np.ndarray, _trace: bool = False) -> np.ndarray:
    global LAST_RESULTS
    nc = _get_program()

    x = np.concatenate([np.asarray(z_i), np.asarray(z_j)], axis=0)
    assert x.shape == (N, D) and x.dtype == np.float32
    xn = x / np.maximum(np.sqrt((x.astype(np.float64) ** 2).sum(axis=1,
                        keepdims=True)), 1e-8)
    x8 = (xn * F8SCALE).astype(F8NP)             # [8192, 1024] fp8
    x8t = np.ascontiguousarray(x8.T)             # [1024, 8192] fp8

    in_maps = []
    for c in range(NCORES):
        b = ((c + 4) % NCORES) * RPC
        if c < 4:
            g2cols = b + np.arange(NG2)
        else:
            g2cols = b + (np.arange(NG2) + 512) % NG2
        cols = np.concatenate([(c * RPC + np.arange(NREG)) % N, g2cols])
        # chunk-major staging: [NCH, 128, KT, CHUNK], contiguous per
        # (chunk, partition) so each chunk DMA is one 4KB run/partition
        xt_c = x8t[:, cols].reshape(KT, 128, NCH, CHUNK)
        xall_c = np.ascontiguousarray(xt_c.transpose(2, 1, 0, 3))
        in_maps.append({"xall8": xall_c})

    res = run_bass_kernel_spmd(
        nc, in_maps, core_ids=list(range(NCORES)), trace=_trace,
    )
    LAST_RESULTS = res

    # Host epilogue (O(N) numpy, f64): combine row partials with the
    # symmetric colsum partials; numerator + diagonal from the same fp8
    # operands the device multiplies.
    x8f = x8.astype(np.float64)
    ediag = np.exp(EXPSCALE * (x8f * x8f).sum(axis=1))
    idx = np.arange(N)
    etarg = np.exp(EXPSCALE * (x8f * x8f[idx ^ 1]).sum(axis=1))

    denom = np.zeros(N, dtype=np.float64)
    pm = (128 * np.arange(MT)[None, :] + np.arange(128)[:, None]).ravel()
    for c in range(NCORES):
        r = res.results[c]
        rows = RPC * c + pm
        denom[rows] += r["esum"].astype(np.float64).sum(axis=2).ravel()
        cs = r["colsums"].astype(np.float64).sum(axis=0)
        s = NCS * CHUNK
        gcols = (RPC * c + RPC + np.arange(NCS * CHUNK)) % N
        denom[gcols] += cs[0:s]
        b = ((c + 4) % NCORES) * RPC
        if c < 4:
            g2cols = b + np.arange(NG2)
        else:
            g2cols = b + (np.arange(NG2) + 512) % NG2
        denom[g2cols[0:CHUNK]] += cs[s:s + CHUNK]
        denom[g2cols[CHUNK:]] += cs[s + CHUNK:s + NG2]
        denom[c * RPC + CHUNK + np.arange(CHUNK)] += cs[s + NG2:]
    loss = np.mean(np.log(denom - ediag) - np.log(etarg))
    return np.float32(loss)


# revision 16
# speedup vs baseline: 1.0738x; 1.0207x over previous
"""NT-Xent loss on 8 Trainium2 NeuronCores (Bass/Tile), v8.

Reference computation (B=4096, D=1024, T=0.5):
    x  = concat(z_i, z_j)                      # [8192, 1024] f32
    xn = x / ||x||                             # row-normalize
    sim = xn @ xn.T                            # [8192, 8192]
    logits = sim / T, diag masked to -inf
    loss = -mean(log_softmax(logits)[i, target(i)]), target(i) = i ^ 1

The O(N^2 D) work is the softmax DENOMINATOR (row sums of
E = exp(sim/T)); that is what the device computes.  The numerator
E[i, i^1] and the diagonal correction E[i, i] are 8192 dot products
each -- O(N D) -- computed on the host from the very same fp8 operands
the device multiplies, so they agree with the device's matrix entries
to float32 rounding.

E is symmetric, so only half the matrix need be computed.  Core c owns
rows [1024c, 1024(c+1)) and sweeps ~4.25 of the 8 column blocks:
  - blocks c+1..c+3 (mod 8) in full,
  - its own diagonal block minus the lower-left quadrant (the triangle
    trick: quadrant D10 is recovered from a column-sum of D01
    restricted to the top m-half),
  - half of block c+4, quadrant-split between the two endpoint cores
    (cores 0-3 take the diagonal quadrants of their d=4 block, cores
    4-7 the anti-diagonal)
-- together an exact single cover.  Row sums come from the ACT exp
accumulator (wide phases) or GpSimd reduces (1-chunk phases);
transposed halves are covered by DVE column-sum accumulators shipped
to the host.  The host combines partials in f64: an O(N) epilogue.

Normalization happens on the host (O(N*D) staging prep, like the
transpose + fp8 casts); both matmul sides are fp8 at scale 16, sliced
from the same chunk tiles (the d=0 chunks ARE the own rows), so the
device is a pure DoubleRow-fp8 sweep (PE) + one wide exp per PSUM tile
(ACT) + bf16 colsum adds (DVE) + rowsum reduces (GpSimd).

Phase structure (chunk-count-increasing so each phase's rhs lands
during earlier phases):
  A: chunk 0, m 0-3 as one quad     (one [128,2048] exp)
  B: chunk 1, two m-quads
  E: chunk 8/9 by m-half, two quads
  C: chunks 2-3 per m               ([128,1024] exps, ACT accum)
  D1: chunks 4-5 per m
  D2: chunks 6-7 per m
1-chunk phases batch 4 m-tiles into one 4-bank PSUM tile so the ACT
chain never gates the PSUM ring; their per-m rowsums cannot use the
ACT accumulator (it would mix the quad's rows), so GpSimd -- otherwise
idle -- reduces each esb quarter (v7 lesson: those reduces plus the
diag/target extraction overloaded the DVE, backed up the 3-deep esb
ring, and stalled ACT -> PSUM -> PE for 3.6 us; v8 drops extraction
entirely, moves reduces to GpSimd, and deepens the esb ring).  Each
colsum accumulator DMAs out as soon as its last add retires; dummy
warm-up matmuls hold the HAM clock-gate at 2.4 GHz through the DMA
prologue.
"""

import numpy as np
import ml_dtypes
from contextlib import ExitStack

import concourse.bass as bass
import concourse.tile as tile
from concourse import bacc, mybir
from concourse.bass_utils import run_bass_kernel_spmd

F32 = mybir.dt.float32
BF16 = mybir.dt.bfloat16
F8 = mybir.dt.float8e4
BF = ml_dtypes.bfloat16
F8NP = ml_dtypes.float8_e4m3
F8SCALE = 16.0
EXPSCALE = 2.0 / (F8SCALE * F8SCALE)   # exp((16 xn_r . 16 xn_c) / (256 T))

B = 4096
D = 1024
N = 2 * B            # 8192 rows total
NCORES = 8
RPC = N // NCORES    # 1024 rows per core
KT = D // 128        # 8 contraction partition-tiles
MT = RPC // 128      # 8 row tiles per core
CHUNK = 512
NREG = 4 * RPC       # regular swept columns (blocks d=0..3)
NG2 = RPC            # staged d=4 columns (chunk 8 for m<4, chunk 9 for m>=4)
NCOL = NREG + NG2    # 5120 staged columns
NCH = NCOL // CHUNK  # 10 column chunks
NCS = 6              # chunks whose colsums ship (d=1..3)
NWARM = 11           # dummy PE warm-up matmuls during the DMA prologue

# phases: (chunk list, m-groups); -1 means chunk 8 or 9 by m-half.
# 1-chunk phases use m-quads (4 m-tiles per PSUM tile); wide phases one
# m per tile.  Phase A covers only the top m-half of chunk 0 (triangle).
PHASES = [
    ([0], [[0, 1, 2, 3]]),
    ([1], [[0, 1, 2, 3], [4, 5, 6, 7]]),
    ([-1], [[0, 1, 2, 3], [4, 5, 6, 7]]),
    ([2, 3], [[m] for m in range(MT)]),
    ([4, 5], [[m] for m in range(MT)]),
    ([6, 7], [[m] for m in range(MT)]),
]
NPH = len(PHASES)

_NC_CACHE = {}
LAST_RESULTS = None  # BassKernelResults of the most recent run (for test.py)


def _build_program():
    nc = bacc.Bacc("TRN2", target_bir_lowering=False, debug=False)

    xall8 = nc.dram_tensor("xall8", [NCH, 128, KT, CHUNK], F8, kind="ExternalInput")
    esum_out = nc.dram_tensor("esum", [128, MT, NPH], F32, kind="ExternalOutput")
    # [cs13 d=1..3 | cs4a | cs4b | cs_d01]
    cs_out = nc.dram_tensor("colsums", [128, NCS * CHUNK + NG2 + CHUNK], BF16,
                            kind="ExternalOutput")

    ADD = mybir.AluOpType.add
    EXP = mybir.ActivationFunctionType.Exp

    with tile.TileContext(nc) as tc, ExitStack() as ctx:
        consts = ctx.enter_context(tc.tile_pool(name="consts", bufs=1))
        xin_pool = ctx.enter_context(tc.tile_pool(name="xin", bufs=1))
        exp_pool = ctx.enter_context(tc.tile_pool(name="exp", bufs=8))
        stat_pool = ctx.enter_context(tc.tile_pool(name="stat", bufs=1))
        ps_pool = ctx.enter_context(tc.tile_pool(name="ps", bufs=2, space="PSUM"))
        # 2 x 4-bank PSUM tiles == all 8 banks

        # PE warm-up operand: memset'd zeros, no DMA dependency
        zbf = consts.tile([128, 640], BF16)
        nc.vector.memset(zbf[:], 0.0)

        xc = [xin_pool.tile([128, KT, CHUNK], F8, name=f"xc{j}")
              for j in range(NCH)]

        # DMA issue order by first use: chunk 0 split across the three
        # DMA-capable queues, chunk 1 halved, then chunks 8/9 (phase E
        # is third), then the rest round-robin.
        nc.scalar.dma_start(xc[0][:, 0:3], xall8[0, :, 0:3])
        nc.sync.dma_start(xc[0][:, 3:6], xall8[0, :, 3:6])
        nc.gpsimd.dma_start(xc[0][:, 6:KT], xall8[0, :, 6:KT])
        nc.scalar.dma_start(xc[1][:, 0:3], xall8[1, :, 0:3])
        nc.sync.dma_start(xc[1][:, 3:6], xall8[1, :, 3:6])
        nc.gpsimd.dma_start(xc[1][:, 6:KT], xall8[1, :, 6:KT])
        nc.gpsimd.dma_start(xc[8][:], xall8[8])
        nc.scalar.dma_start(xc[9][:], xall8[9])
        nc.sync.dma_start(xc[2][:], xall8[2])
        nc.gpsimd.dma_start(xc[3][:], xall8[3])
        nc.scalar.dma_start(xc[4][:], xall8[4])
        nc.sync.dma_start(xc[5][:], xall8[5])
        nc.gpsimd.dma_start(xc[6][:], xall8[6])
        nc.scalar.dma_start(xc[7][:], xall8[7])

        esum = stat_pool.tile([128, MT, NPH], F32)
        cs13 = stat_pool.tile([128, NCS * CHUNK], BF16)
        cs4a = stat_pool.tile([128, CHUNK], BF16)
        cs4b = stat_pool.tile([128, CHUNK], BF16)
        csd01 = stat_pool.tile([128, CHUNK], BF16)
        nc.vector.memset(esum[:], 0.0)
        nc.vector.memset(cs13[:], 0.0)
        nc.vector.memset(cs4a[:], 0.0)
        nc.vector.memset(cs4b[:], 0.0)
        nc.vector.memset(csd01[:], 0.0)

        # HAM warm-up: dummy bf16 matmuls fill the otherwise-idle PE
        # during the first chunk's DMA so the 2.4 GHz clock-gate opens
        # before the real sweep starts.
        ps_w = ps_pool.tile([128, 2048], F32, name="ps", tag="ps")
        for _ in range(NWARM):
            nc.tensor.matmul(ps_w[:, 0:CHUNK], lhsT=zbf[:, 0:128],
                             rhs=zbf[:, 128:640], start=True, stop=True)

        for ph, (chunks, mgroups) in enumerate(PHASES):
            for mg in mgroups:
                quad = len(mg) > 1
                nch = len(chunks)
                w = CHUNK * (len(mg) if quad else nch)
                ps = ps_pool.tile([128, 2048], F32, name="ps", tag="ps")
                for idx, m in enumerate(mg):
                    cj = [(8 if m < 4 else 9) if j < 0 else j for j in chunks]
                    for t in range(KT // 2):
                        for ci, j in enumerate(cj):
                            bank = idx if quad else ci
                            nc.tensor.matmul(
                                ps[:, CHUNK * bank:CHUNK * (bank + 1)],
                                lhsT=xc[m // 4][:, 2 * t:2 * t + 2,
                                                128 * (m % 4):128 * (m % 4) + 128],
                                rhs=xc[j][:, 2 * t:2 * t + 2, :],
                                start=(t == 0), stop=(t == KT // 2 - 1),
                                perf_mode=mybir.MatmulPerfMode.DoubleRow,
                            )
                esb = exp_pool.tile([128, w], BF16)
                if quad:
                    nc.scalar.activation(esb[:], ps[:, 0:w], EXP,
                                         scale=EXPSCALE)
                    for idx, m in enumerate(mg):
                        nc.vector.tensor_reduce(
                            esum[:, m, ph:ph + 1],
                            esb[:, CHUNK * idx:CHUNK * (idx + 1)],
                            axis=mybir.AxisListType.X, op=ADD)
                else:
                    m = mg[0]
                    nc.scalar.activation(esb[:], ps[:, 0:w], EXP,
                                         scale=EXPSCALE,
                                         accum_out=esum[:, m, ph:ph + 1])
                for idx, m in enumerate(mg):
                    base = CHUNK * idx if quad else 0
                    if ph == 1 and m < 4:
                        nc.gpsimd.tensor_add(csd01[:], csd01[:],
                                             esb[:, base:base + CHUNK])
                    elif ph == 2:
                        tgt = cs4a if m < 4 else cs4b
                        nc.gpsimd.tensor_add(tgt[:], tgt[:],
                                             esb[:, base:base + CHUNK])
                    elif ph == 3:
                        nc.vector.tensor_add(cs13[:, 0:1024],
                                             cs13[:, 0:1024], esb[:])
                    elif ph == 4:
                        nc.vector.tensor_add(cs13[:, 1024:2048],
                                             cs13[:, 1024:2048], esb[:])
                    elif ph == 5:
                        nc.vector.tensor_add(cs13[:, 2048:3072],
                                             cs13[:, 2048:3072], esb[:])
            # ship each accumulator as soon as its last add retires so
            # only the final phase's piece remains for the tail
            s = NCS * CHUNK
            if ph == 1:
                nc.sync.dma_start(cs_out[:, s + 2 * CHUNK:], csd01[:])
            elif ph == 2:
                nc.sync.dma_start(cs_out[:, s:s + CHUNK], cs4a[:])
                nc.sync.dma_start(cs_out[:, s + CHUNK:s + 2 * CHUNK],
                                  cs4b[:])
            elif ph == 3:
                nc.sync.dma_start(cs_out[:, 0:1024], cs13[:, 0:1024])
            elif ph == 4:
                nc.sync.dma_start(cs_out[:, 1024:2048], cs13[:, 1024:2048])

        nc.sync.dma_start(cs_out[:, 2048:2560], cs13[:, 2048:2560])
        nc.scalar.dma_start(cs_out[:, 2560:3072], cs13[:, 2560:3072])
        nc.sync.dma_start(esum_out[:], esum[:])

    nc.finalize()
    return nc


def _get_program():
    if "nc" not in _NC_CACHE:
        _NC_CACHE["nc"] = _build_program()
    return _NC_CACHE["nc"]


def kernel(z_i: np.ndarray, z_j: np.ndarray, _trace: bool = False) -> np.ndarray:
    global LAST_RESULTS
    nc = _get_program()

    x = np.concatenate([np.asarray(z_i), np.asarray(z_j)], axis=0)
    assert x.shape == (N, D) and x.dtype == np.float32
    xn = x / np.maximum(np.sqrt((x.astype(np.float64) ** 2).sum(axis=1,
                        keepdims=True)), 1e-8)
    x8 = (xn * F8SCALE).astype(F8NP)             # [8192, 1024] fp8
    x8t = np.ascontiguousarray(x8.T)             # [1024, 8192] fp8

    in_maps = []
    for c in range(NCORES):
        b = ((c + 4) % NCORES) * RPC
        if c < 4:
            g2cols = b + np.arange(NG2)
        else:
            g2cols = b + (np.arange(NG2) + 512) % NG2
        cols = np.concatenate([(c * RPC + np.arange(NREG)) % N, g2cols])
        # chunk-major staging: [NCH, 128, KT, CHUNK], contiguous per
        # (chunk, partition) so each chunk DMA is one 4KB run/partition
        xt_c = x8t[:, cols].reshape(KT, 128, NCH, CHUNK)
        xall_c = np.ascontiguousarray(xt_c.transpose(2, 1, 0, 3))
        in_maps.append({"xall8": xall_c})

    res = run_bass_kernel_spmd(
        nc, in_maps, core_ids=list(range(NCORES)), trace=_trace,
    )
    LAST_RESULTS = res

    # Host epilogue (O(N) numpy, f64): combine row partials with the
    # symmetric colsum partials; numerator + diagonal from the same fp8
    # operands the device multiplies.
    x8f = x8.astype(np.float64)
    ediag = np.exp(EXPSCALE * (x8f * x8f).sum(axis=1))
    idx = np.arange(N)
    etarg = np.exp(EXPSCALE * (x8f * x8f[idx ^ 1]).sum(axis=1))

    denom = np.zeros(N, dtype=np.float64)
    pm = (128 * np.arange(MT)[None, :] + np.arange(128)[:, None]).ravel()
    for c in range(NCORES):
        r = res.results[c]
        rows = RPC * c + pm
        denom[rows] += r["esum"].astype(np.float64).sum(axis=2).ravel()
        cs = r["colsums"].astype(np.float64).sum(axis=0)
        s = NCS * CHUNK
        gcols = (RPC * c + RPC + np.arange(NCS * CHUNK)) % N
        denom[gcols] += cs[0:s]
        b = ((c + 4) % NCORES) * RPC
        if c < 4:
            g2cols = b + np.arange(NG2)
        else:
            g2cols = b + (np.arange(NG2) + 512) % NG2
        denom[g2cols[0:CHUNK]] += cs[s:s + CHUNK]
        denom[g2cols[CHUNK:]] += cs[s + CHUNK:s + NG2]
        denom[c * RPC + CHUNK + np.arange(CHUNK)] += cs[s + NG2:]
    loss = np.mean(np.log(denom - ediag) - np.log(etarg))
    return np.float32(loss)
